# revision 1
# baseline (speedup 1.0000x reference)
"""Trainium2 Bass kernel for AttentionAggregate (GAT-style neighbor aggregation).

Reference computation (per node n, neighbors k=0..K-1):
    pt = target @ W.T + b                      # [N, D]
    pm = middle @ W.T + b                      # [N, K, D]
    score = leaky_relu((pt[:,None,:] + pm) @ a_w.T + a_b)
    coef  = softmax(score, axis=K)
    out   = sum_k coef * middle                # [N, D]

Key algebraic simplification: the W-projection only enters through the dot
with a_w, so with u = a_w @ W (a single D-vector) and c = 2*(a_w.b) + a_b:
    score[n,k] = target[n].u + middle[n,k].u + c
This removes all large matmuls; the kernel is a memory-bound pass over
`middle` (512 MiB) with per-node softmax weighting.

Sharding: data-parallel over nodes. N=16384 nodes split across 8 cores
(2048 nodes each); W/b/a_w/a_b replicated; no cross-core communication.

Per 128-node tile [128, K, D] (node on partition):
  phase 1 (DVE): m2 = mid * u_rep (one big TT), s = segmented reduce over d.
  softmax (DVE+ACT): add target.u + c via tensor_scalar (two per-partition
      scalars), leaky = max(x, 0.01x), exp(bias=-max), reciprocal, scale.
  phase 2 (PE): out = sum_k diag(coef[:,k]) @ mid[:,k,:] — 32 accumulating
      matmuls whose stationary operand is the identity scaled per-partition
      by coef[:,k] (a per-partition scale on the PE). ACT evacuates PSUM.
"""

from contextlib import ExitStack

import numpy as np

import concourse.bass as bass
import concourse.tile as tile
from concourse import mybir
from concourse.bass_utils import run_bass_kernel_spmd

N_CORES = 8
N, K, D = 16384, 32, 256
NS = N // N_CORES  # nodes per core
P = 128
F32 = mybir.dt.float32
ALU = mybir.AluOpType
AF = mybir.ActivationFunctionType
AX = mybir.AxisListType
NEG_SLOPE = 0.01


def emit_kernel(tc, out, tgt, mid, W, b, a_w, a_b, ident, ns):
    nc = tc.nc
    nt = ns // P  # node tiles per core
    with ExitStack() as ctx:
        singles = ctx.enter_context(tc.tile_pool(name="singles", bufs=1))
        mids = ctx.enter_context(tc.tile_pool(name="mids", bufs=2))
        small = ctx.enter_context(tc.tile_pool(name="small", bufs=3))
        scr = ctx.enter_context(tc.tile_pool(name="scr", bufs=2))
        diags = ctx.enter_context(tc.tile_pool(name="diags", bufs=4))
        psum = ctx.enter_context(tc.tile_pool(name="psum", bufs=2, space="PSUM"))
        outs = ctx.enter_context(tc.tile_pool(name="outs", bufs=3))

        # ---- setup: u = a_w @ W, c = 2*(a_w.b) + a_b ----
        W0 = singles.tile([P, D], F32)
        W1 = singles.tile([P, D], F32)
        nc.sync.dma_start(W0, W[0:P, :])
        nc.sync.dma_start(W1, W[P : 2 * P, :])
        # a_w transposed onto partitions: awT[p, g] = a_w[0, g*128 + p]
        awT = singles.tile([P, 2], F32)
        nc.sync.dma_start(awT, a_w.rearrange("o (g p) -> p (g o)", g=2))
        b_row = singles.tile([1, D], F32)
        nc.sync.dma_start(b_row, b.unsqueeze(0))
        aw_row = singles.tile([1, D], F32)
        nc.sync.dma_start(aw_row, a_w)
        ab_t = singles.tile([1, 1], F32)
        nc.sync.dma_start(ab_t, a_b.unsqueeze(0))
        id_t = singles.tile([P, P], F32)
        nc.sync.dma_start(id_t, ident)

        # Wsc[d, e] = a_w[d] * W[d, e]
        Wsc0 = singles.tile([P, D], F32)
        Wsc1 = singles.tile([P, D], F32)
        nc.vector.tensor_scalar_mul(Wsc0, W0, awT[:, 0:1])
        nc.vector.tensor_scalar_mul(Wsc1, W1, awT[:, 1:2])
        ones_col = singles.tile([P, 1], F32)
        ones_row = singles.tile([1, P], F32)
        nc.vector.memset(ones_col, 1.0)
        nc.vector.memset(ones_row, 1.0)
        # u[e] = sum_d Wsc[d, e]  (partition reduction via PE)
        u_ps = psum.tile([1, D], F32)
        nc.tensor.matmul(u_ps, ones_col, Wsc0, start=True, stop=False)
        nc.tensor.matmul(u_ps, ones_col, Wsc1, start=False, stop=True)
        u_row = singles.tile([1, D], F32)
        nc.scalar.copy(u_row, u_ps)

        # c = 2*(b . a_w) + a_b
        baw_scr = small.tile([1, D], F32, tag="baw_scr")
        baw = small.tile([1, 1], F32, tag="baw")
        nc.vector.tensor_mul(baw_scr, b_row, aw_row)
        nc.vector.reduce_sum(baw, baw_scr, AX.X)
        c_s = singles.tile([1, 1], F32)
        nc.scalar.activation(c_s, baw, AF.Identity, bias=ab_t, scale=2.0)

        # broadcast u, c across all 128 partitions via PE outer product
        ub_ps = psum.tile([P, D], F32)
        nc.tensor.matmul(ub_ps, ones_row, u_row, start=True, stop=True)
        u_b = singles.tile([P, D], F32)
        nc.scalar.copy(u_b, ub_ps)
        cb_ps = psum.tile([P, 1], F32)
        nc.tensor.matmul(cb_ps, ones_row, c_s, start=True, stop=True)
        c_b = singles.tile([P, 1], F32)
        nc.scalar.copy(c_b, cb_ps)

        # u replicated K times along free dim for the big per-tile multiply
        u_rep = singles.tile([P, K, D], F32)
        for k in range(K):
            nc.vector.tensor_copy(u_rep[:, k, :], u_b)

        # ---- stc[:, t] = target[t*128:(t+1)*128] . u  (c added later) ----
        stc = singles.tile([P, nt], F32)
        for t in range(nt):
            tg = small.tile([P, D], F32, tag="tg")
            nc.sync.dma_start(tg, tgt[t * P : (t + 1) * P, :])
            pscr = scr.tile([P, D], F32, tag="tscr")
            nc.vector.tensor_mul(pscr, tg, u_b)
            nc.vector.reduce_sum(stc[:, t : t + 1], pscr, AX.X)

        # ---- main loop over node tiles ----
        for t in range(nt):
            m = mids.tile([P, K, D], F32, tag="mid")
            nc.sync.dma_start(m, mid[t * P : (t + 1) * P, :, :])

            # phase 1: m2 = mid * u_rep; s[:, k] = sum_d m2[:, k, :]
            m2 = scr.tile([P, K, D], F32, tag="m2")
            nc.vector.tensor_mul(m2, m, u_rep)
            s = small.tile([P, K], F32, tag="s")
            nc.vector.reduce_sum(s, m2, AX.X)

            # scores: sb = s + stc[:,t] + c; leaky = max(sb, 0.01*sb)
            sb = small.tile([P, K], F32, tag="sb")
            nc.vector.tensor_scalar(
                out=sb, in0=s, scalar1=stc[:, t : t + 1], scalar2=c_b,
                op0=ALU.add, op1=ALU.add,
            )
            s01 = small.tile([P, K], F32, tag="s01")
            nc.vector.tensor_scalar_mul(s01, sb, NEG_SLOPE)
            s2 = small.tile([P, K], F32, tag="s2")
            nc.vector.tensor_max(s2, sb, s01)

            # softmax over k
            mx = small.tile([P, 1], F32, tag="mx")
            nc.vector.reduce_max(mx, s2, AX.X)
            nmx = small.tile([P, 1], F32, tag="nmx")
            nc.vector.tensor_scalar_mul(nmx, mx, -1.0)
            e = small.tile([P, K], F32, tag="e")
            nc.scalar.activation(e, s2, AF.Exp, bias=nmx, scale=1.0)
            den = small.tile([P, 1], F32, tag="den")
            nc.vector.reduce_sum(den, e, AX.X)
            rcp = small.tile([P, 1], F32, tag="rcp")
            nc.vector.reciprocal(rcp, den)
            coef = small.tile([P, K], F32, tag="coef")
            nc.vector.tensor_scalar_mul(coef, e, rcp)

            # phase 2 on PE: out = sum_k diag(coef[:,k]) @ mid[:,k,:]
            # diag(coef_k) = identity scaled per-partition by coef[:,k].
            o_ps = psum.tile([P, D], F32, tag="o_ps")
            for k in range(K):
                dg = diags.tile([P, P], F32, tag="dg")
                nc.vector.tensor_scalar_mul(dg, id_t, coef[:, k : k + 1])
                nc.tensor.matmul(
                    o_ps, dg, m[:, k, :],
                    start=(k == 0), stop=(k == K - 1), skip_group_check=True,
                )
            o_sb = outs.tile([P, D], F32, tag="o_sb")
            nc.scalar.copy(o_sb, o_ps)
            nc.sync.dma_start(out[t * P : (t + 1) * P, :], o_sb)


def build_nc(ns=NS):
    nc = bass.Bass("TRN2", debug=False, num_devices=N_CORES)
    tgt = nc.dram_tensor("target", [ns, D], F32, kind="ExternalInput").ap()
    mid = nc.dram_tensor("middle", [ns, K, D], F32, kind="ExternalInput").ap()
    W = nc.dram_tensor("W", [D, D], F32, kind="ExternalInput").ap()
    b = nc.dram_tensor("b", [D], F32, kind="ExternalInput").ap()
    a_w = nc.dram_tensor("a_w", [1, D], F32, kind="ExternalInput").ap()
    a_b = nc.dram_tensor("a_b", [1], F32, kind="ExternalInput").ap()
    ident = nc.dram_tensor("ident", [P, P], F32, kind="ExternalInput").ap()
    out = nc.dram_tensor("out", [ns, D], F32, kind="ExternalOutput").ap()
    with tile.TileContext(nc) as tc:
        emit_kernel(tc, out, tgt, mid, W, b, a_w, a_b, ident, ns)
    import bass_rust as _br

    # Split multi-wait instructions (walrus allows at most 1 sync wait per
    # instruction; Tile can emit more after multi-DMA dependencies).
    _br.generate_event_semaphores(nc)
    return nc


_NC_CACHE = {}


def _get_nc(ns=NS):
    if ns not in _NC_CACHE:
        _NC_CACHE[ns] = build_nc(ns)
    return _NC_CACHE[ns]


def make_in_maps(target, middle, W, b, a_w, a_b):
    target = np.ascontiguousarray(np.asarray(target, dtype=np.float32))
    middle = np.ascontiguousarray(np.asarray(middle, dtype=np.float32))
    W = np.ascontiguousarray(np.asarray(W, dtype=np.float32))
    b = np.ascontiguousarray(np.asarray(b, dtype=np.float32))
    a_w = np.ascontiguousarray(np.asarray(a_w, dtype=np.float32))
    a_b = np.ascontiguousarray(np.asarray(a_b, dtype=np.float32))
    ident = np.eye(P, dtype=np.float32)
    tgt_shards = np.split(target, N_CORES, axis=0)
    mid_shards = np.split(middle, N_CORES, axis=0)
    return [
        {
            "target": tgt_shards[i],
            "middle": mid_shards[i],
            "W": W,
            "b": b,
            "a_w": a_w,
            "a_b": a_b,
            "ident": ident,
        }
        for i in range(N_CORES)
    ]


def run_sharded(in_maps, **kwargs):
    nc = _get_nc(in_maps[0]["target"].shape[0])
    res = run_bass_kernel_spmd(nc, in_maps, list(range(N_CORES)), **kwargs)
    full = np.concatenate([r["out"] for r in res.results], axis=0)
    return full, res


def kernel(target, middle, W, b, a_w, a_b):
    in_maps = make_in_maps(target, middle, W, b, a_w, a_b)
    full, _ = run_sharded(in_maps)
    return full

